# revision 8
# baseline (speedup 1.0000x reference)
"""Trainium2 Bass kernel for a single-head causal attention module.

Problem (hardcoded): x [8, 2048, 1024] f32, W_Q/W_K/W_V [64, 1024] f32
    Q = x @ W_Q.T ; K = x @ W_K.T ; V = x @ W_V.T       (per batch)
    out = softmax(causal(Q @ K.T / sqrt(64))) @ V        -> [8, 2048, 64] f32

Sharding: batch dim across the 8 NeuronCores (data parallel, no collectives).

v3 dataflow (bf16 compute, fp32 PSUM accumulation for all real matmuls):
  - Host casts x and the packed weights to bf16 (tolerance is 2e-2; measured
    end-to-end error of the bf16 pipeline is ~3e-3). Halves the x DMA bytes.
  - x^T via PE transposes in bf16 (1 cy/row vs 2 for f32): 32 tile transposes
    per strip into a 1-bank bf16 PSUM tile, drained to SBUF by VectorE.
  - QKV projections contract d in 8 chain-matmuls per 512-wide s strip;
    W_Q^T|W_K^T pack the stationary so Q^T and K^T come out of one chain,
    j-major, which is exactly what the scores matmul streams.
  - V^T -> s-major [V|1] via ONE xbar DMA-transpose per strip (bf16-only HW
    feature, runs on the otherwise idle Sync DGE): V^T is padded to 80 rows
    (xbar wants multiples of 16) with row 64 all-ones, so the transposed
    stationary directly yields softmax row sums during P@[V|1].
  - Attention per 1024-wide q half: per key tile a 512-col-pair of scores
    matmuls (64-deep bf16 contraction, no padding; the pair shares its K^T
    stationary), ONE fused exp(scale=0.125) on ScalarE producing bf16 P^T,
    an exact 0/1 triu multiply on the diagonal block only, then P^T@[V|1]
    accumulates per-chunk O^T in f32 PSUM.
  - ScalarE runs ONLY the exps (activation-table thrash avoided); all
    copies/casts are pinned to VectorE/GpSimd.
  - Retired q chunks are normalized via PE transpose + reciprocal row-sum
    scale into [128, 64] f32 output tiles (output stays f32).
  - identity/triu are DRAM constants so no engine work gates PE warmup.
  - PSUM (8 banks): transpose 1 + proj 1 + scores 2x2 + 2 O^T accumulators,
    so attention halves fully overlap the neighbouring strip builds.
"""

import numpy as np
import ml_dtypes

import concourse.mybir as mybir
import concourse.tile as tile
from concourse import bacc
from concourse.bass_utils import run_bass_kernel_spmd

B, S, D, J, P = 8, 2048, 1024, 64, 128
NCH = D // P  # 8 contraction chunks of 128
NSG = 4  # 512-wide s/q strips
SW = S // NSG  # 512
NT = S // P  # 16 key tiles
HW_ = 1024  # attention half-strip width
VP = 80  # V^T rows padded to x16 for the xbar transpose; row 64 = ones
F32 = mybir.dt.float32
BF16 = mybir.dt.bfloat16
NWARM = 16  # fp32 warmup matmuls to ramp the HAM clock while DMAs land


def _build():
    nc = bacc.Bacc("TRN2", debug=False)
    x = nc.dram_tensor("x", [S, D], BF16, kind="ExternalInput").ap()
    wqk = nc.dram_tensor("WQK", [D, P], BF16, kind="ExternalInput").ap()
    wv = nc.dram_tensor("WV", [D, J], BF16, kind="ExternalInput").ap()
    ident_d = nc.dram_tensor("IDENT", [P, P], F32, kind="ExternalInput").ap()
    identb_d = nc.dram_tensor("IDENTB", [P, P], BF16, kind="ExternalInput").ap()
    triu_d = nc.dram_tensor("TRIU", [P, P], BF16, kind="ExternalInput").ap()
    out = nc.dram_tensor("out", [S, J], F32, kind="ExternalOutput").ap()

    AF = mybir.ActivationFunctionType

    with tile.TileContext(nc) as tc:
        from contextlib import ExitStack

        with ExitStack() as ctx:
            persist = ctx.enter_context(tc.tile_pool(name="persist", bufs=1))
            xsb_pool = ctx.enter_context(tc.tile_pool(name="xsb", bufs=3))
            pt_pool = ctx.enter_context(tc.tile_pool(name="ptp", bufs=4))
            otsb_pool = ctx.enter_context(tc.tile_pool(name="otsb", bufs=2))
            osb_pool = ctx.enter_context(tc.tile_pool(name="osb", bufs=3))
            rcp_pool = ctx.enter_context(tc.tile_pool(name="rcp", bufs=3))
            # PSUM budget (8 banks): tp 1 (bf16 x^T staging) + proj 1 +
            # sc x2 (2 banks each) + 2 O^T accumulators.
            pstp = ctx.enter_context(tc.tile_pool(name="pstp", bufs=1, space="PSUM"))
            psA = ctx.enter_context(tc.tile_pool(name="psA", bufs=1, space="PSUM"))
            pssc = ctx.enter_context(tc.tile_pool(name="pssc", bufs=2, space="PSUM"))
            psot = ctx.enter_context(tc.tile_pool(name="psot", bufs=1, space="PSUM"))

            ident = persist.tile([P, P], F32, tag="ident")
            nc.sync.dma_start(ident, ident_d)
            identb = persist.tile([P, P], BF16, tag="identb")
            nc.sync.dma_start(identb, identb_d)
            triu = persist.tile([P, P], BF16, tag="triu")
            nc.sync.dma_start(triu, triu_d)

            wqk_t = persist.tile([P, NCH, P], BF16, tag="wqkt")
            wv_t = persist.tile([P, NCH, J], BF16, tag="wvt")
            nc.sync.dma_start(wqk_t, wqk.rearrange("(c p) m -> p c m", p=P))
            nc.sync.dma_start(wv_t, wv.rearrange("(c p) m -> p c m", p=P))

            x_r = x.rearrange("(t p) d -> p t d", p=P)  # [128, 16, 1024]

            xt = persist.tile([P, NCH, S], BF16, tag="xt")
            qt = persist.tile([J, S], BF16, tag="qt")
            kt = persist.tile([J, S], BF16, tag="kt")
            vt = persist.tile([VP, S], BF16, tag="vt")
            nc.gpsimd.memset(vt[J:VP, :], 1.0)
            # V s-major per key tile: [:, t, 0:64] = V, [:, t, 64] = ones
            vaug = persist.tile([P, NT, VP], BF16, tag="vaug")

            # PE warmup: the HAM clock gate needs ~3.4us of sustained matmul
            # activity to unthrottle 1.2 -> 2.4 GHz; spin on the identity
            # (64KB DMA, lands almost immediately) while x strips stream in.
            pswu = psA.tile([P, SW], F32, tag="proj", name="pswu")
            for i in range(NWARM):
                nc.tensor.matmul(
                    pswu[:, 0:P],
                    ident,
                    ident,
                    start=(i == 0),
                    stop=(i == NWARM - 1),
                )

            out_r = out.rearrange("(t p) j -> p t j", p=P)  # [128, 16, 64]

            def dma_strip(g):
                xs = xsb_pool.tile([P, 4, D], BF16, tag="xs", name="xs")
                for half in range(2):  # split DMA so transposes start earlier
                    nc.sync.dma_start(
                        xs[:, 2 * half : 2 * half + 2, :],
                        x_r[:, 4 * g + 2 * half : 4 * g + 2 * half + 2, :],
                    )
                return xs

            def build_strip(g, xs):
                """Transpose strip g, project Q^T|K^T and V^T, build [V|1]."""
                sl = slice(SW * g, SW * (g + 1))
                for grp in range(NCH // 2):
                    pst = pstp.tile([P, 2, SW], BF16, tag="tp", name="pst")
                    for sub in range(2):
                        c = 2 * grp + sub
                        for k in range(4):
                            nc.tensor.transpose(
                                pst[:, sub, P * k : P * k + P],
                                xs[:, k, P * c : P * c + P],
                                identb,
                            )
                    nc.vector.tensor_copy(xt[:, 2 * grp : 2 * grp + 2, sl], pst)
                psqk = psA.tile([P, SW], F32, tag="proj", name="psqk")
                for dc in range(NCH):
                    nc.tensor.matmul(
                        psqk,
                        wqk_t[:, dc, :],
                        xt[:, dc, sl],
                        start=(dc == 0),
                        stop=(dc == NCH - 1),
                    )
                nc.vector.tensor_copy(qt[:, sl], psqk[0:J])
                nc.vector.tensor_copy(kt[:, sl], psqk[J:P])
                psv = psA.tile([P, SW], F32, tag="proj", name="psv")
                for dc in range(NCH):
                    nc.tensor.matmul(
                        psv[0:J],
                        wv_t[:, dc, :],
                        xt[:, dc, sl],
                        start=(dc == 0),
                        stop=(dc == NCH - 1),
                    )
                nc.vector.tensor_copy(vt[0:J, sl], psv[0:J])
                nc.sync.dma_start_transpose(
                    vaug[:, 4 * g : 4 * (g + 1), :], vt[:, sl]
                )

            def finalize_chunk(c, ot):
                """Normalize O^T chunk c and write [128, 64] output tiles."""
                otsb = otsb_pool.tile([J + 1, SW], F32, tag="otsb", name="otsb")
                nc.vector.tensor_copy(otsb, ot)  # gpsimd has no PSUM port
                # odd chunks: the other accumulator slot is free too, so the
                # four transposes double-buffer across both ot banks
                tags = ("ot0", "ot1") if c % 2 else (f"ot{c % 2}",)
                o = osb_pool.tile([P, 4, J], F32, tag="o", name="o")
                for k in range(4):
                    pso = psot.tile([P, 72], F32, tag=tags[k % len(tags)], name="pso")
                    nc.tensor.transpose(
                        pso[:, 0 : J + 1],
                        otsb[:, P * k : P * k + P],
                        ident[0 : J + 1, 0 : J + 1],
                    )
                    rc = rcp_pool.tile([P, 1], F32, tag="rc", name="rc")
                    nc.vector.reciprocal(rc, pso[:, J : J + 1])
                    nc.vector.tensor_scalar_mul(out=o[:, k, :], in0=pso[:, 0:J], scalar1=rc)
                    if c == NSG - 1 and k % 2:  # drain the tail DMA early
                        nc.sync.dma_start(
                            out_r[:, 4 * c + k - 1 : 4 * c + k + 1, :],
                            o[:, k - 1 : k + 1, :],
                        )
                if c != NSG - 1:
                    nc.sync.dma_start(out_r[:, 4 * c : 4 * c + 4, :], o)

            def attn_half(h):
                """Scores/softmax/PV for q in [1024h, 1024h+1024)."""
                ot = {
                    c: psot.tile([J + 1, SW], F32, tag=f"ot{c % 2}", name="ot")
                    for c in (2 * h, 2 * h + 1)
                }
                q0 = HW_ * h
                for t in range(8 * h + 8):
                    off = max(0, P * t - q0)
                    pssh = pssc.tile([P, HW_], F32, tag="sc", name="pssh")
                    # two matmuls: PSUM accumulation groups cannot span banks
                    # (512 f32); the pair shares its K^T stationary.
                    for lo2 in (0, SW):
                        o2 = max(off, lo2)
                        if o2 >= lo2 + SW:
                            continue
                        nc.tensor.matmul(
                            pssh[:, o2 : lo2 + SW],
                            kt[:, P * t : P * t + P],
                            qt[:, q0 + o2 : q0 + lo2 + SW],
                            start=True,
                            stop=True,
                        )
                    ptc = pt_pool.tile([P, HW_], BF16, tag="ptc", name="ptc")
                    nc.scalar.activation(
                        ptc[:, off:HW_], pssh[:, off:HW_], AF.Exp, scale=0.125
                    )
                    if t // 8 == h:  # diagonal block lives in this half
                        nc.vector.tensor_mul(
                            ptc[:, off : off + P], ptc[:, off : off + P], triu
                        )
                    for c in (2 * h, 2 * h + 1):
                        if t > 4 * c + 3:
                            continue
                        lo = SW * c - q0
                        co = max(0, P * t - SW * c)
                        nc.tensor.matmul(
                            ot[c][:, co:SW],
                            vaug[:, t, 0 : J + 1],
                            ptc[:, lo + co : lo + SW],
                            start=(t == 0),
                            stop=(t == 4 * c + 3),
                        )
                        if t == 4 * c + 3:
                            finalize_chunk(c, ot[c])

            xs0 = dma_strip(0)
            xs1 = dma_strip(1)
            build_strip(0, xs0)
            build_strip(1, xs1)
            attn_half(0)  # only needs strips 0-1; overlaps strips 2-3 below
            build_strip(2, dma_strip(2))
            build_strip(3, dma_strip(3))
            attn_half(1)

    nc.compile()
    return nc


_NC_CACHE = {}


def _get_nc():
    if "nc" not in _NC_CACHE:
        _NC_CACHE["nc"] = _build()
    return _NC_CACHE["nc"]


def make_in_maps(x, W_Q, W_K, W_V):
    bf16 = ml_dtypes.bfloat16
    x = np.asarray(x, dtype=np.float32)
    W_Q = np.asarray(W_Q, dtype=np.float32)
    W_K = np.asarray(W_K, dtype=np.float32)
    W_V = np.asarray(W_V, dtype=np.float32)
    assert x.shape == (B, S, D)
    # weight layout prep (host, once): [j, d] -> packed d-major [d, j], bf16
    wqk_host = np.ascontiguousarray(
        np.concatenate([W_Q.T, W_K.T], axis=1).astype(bf16)
    )
    wv_host = np.ascontiguousarray(W_V.T.astype(bf16))
    ident_host = np.eye(P, dtype=np.float32)
    identb_host = np.eye(P, dtype=np.float32).astype(bf16)
    triu_host = np.triu(np.ones((P, P), dtype=np.float32)).astype(bf16)
    xb = np.ascontiguousarray(x.astype(bf16))
    return [
        {
            "x": xb[b],
            "WQK": wqk_host,
            "WV": wv_host,
            "IDENT": ident_host,
            "IDENTB": identb_host,
            "TRIU": triu_host,
        }
        for b in range(B)
    ]


def kernel(x, W_Q, W_K, W_V):
    nc = _get_nc()
    in_maps = make_in_maps(x, W_Q, W_K, W_V)
    res = run_bass_kernel_spmd(nc, in_maps, core_ids=list(range(B)))
    return np.stack([r["out"] for r in res.results], axis=0)


if __name__ == "__main__":
    rng = np.random.default_rng(0)
    inputs = {
        "x": rng.standard_normal((B, S, D), dtype=np.float32),
        "W_Q": (rng.random((J, D), dtype=np.float32) - 0.5) / 16.0,
        "W_K": (rng.random((J, D), dtype=np.float32) - 0.5) / 16.0,
        "W_V": (rng.random((J, D), dtype=np.float32) - 0.5) / 16.0,
    }
    got = kernel(**inputs)
    print("out", got.shape, got.dtype, np.abs(got).max())
